# revision 12
# baseline (speedup 1.0000x reference)
"""Trainium2 Bass kernel: 4-bit block-dequant linear  y = x @ dequant(W).T + bias.

Shapes (hardcoded): x[64,4096] f32, weight[11008,2048] int32 (two uint4 nibbles
in the low byte of each int32), scale/zp[11008,1,128] f32, bias[11008] f32.
Output y[64,11008] f32.

Strategy (8-way tensor-parallel over out_features, 1376 rows per core):

  y[b,o] = sum_c x[b,c] * w[o,c] * s[o, c%128]
         - sum_j (zp[o,j]*s[o,j]) * xs[b,j]          (zero-point correction)
         + bias[o]
  where xs[b,j] = sum_i x[b, 128i+j].

Host-side prep: the packed weight is cast to its information content (one
byte per int32, value 0..255, as int16) and laid out per core as 8 pair-groups
of 2x128 partition-rows: wP[p, 2752*g + 1376*q + j] = byte k=256g+128q+p of
out-row j.  On device, per core, 8 pair-groups:
  * one contiguous-per-partition DMA of tb[128, 2752] int16
  * h16 = tb >> 4 (ACT Copy(scale=1/16, bias=-7.5/16) -> int16 rounds = floor)
  * l   = tb & 15 (DVE tensor_scalar 4x-ish, bf16 out)
  * hs = h16 * sce2 (bf16), ls = l * sco2  (DVE 2x / GPSIMD split)
  * PE accumulates into 3 PSUM tiles [64, o-block<=512]:
      bias (K=1) + zp-correction (K=128, rhs = -(zp*s).T)
      + 2x2x3 bf16 matmuls (lhsT = matching x columns, rhs = hs/ls slices)
  * ACT evicts PSUM -> SBUF, DMA to DRAM.

Partition p of pair g / sub q holds byte k = 256g+128q+p = column pair
(i_hi, i_lo) = (2k, 2k+1); i_hi % 128 = 2*(p%64), so the scale tiles
sce[p,o] = s[o, 2(p%64)], sco[p,o] = s[o, 2(p%64)+1] are chunk-independent.
"""

import sys

import numpy as np

for _p in ("/opt/trn_rl_repo", "/root/.axon_site/_ro/trn_rl_repo"):
    if _p not in sys.path:
        sys.path.insert(0, _p)

import ml_dtypes  # noqa: E402
import concourse.bass as bass  # noqa: E402
import concourse.bacc as bacc  # noqa: E402
import concourse.mybir as mybir  # noqa: E402
from concourse import tile  # noqa: E402
from concourse.bass_utils import run_bass_kernel_spmd  # noqa: E402

dt = mybir.dt
Alu = mybir.AluOpType

B = 64
IN = 4096
OUT = 11008
BLK = 128
NCORES = 8
OSH = OUT // NCORES          # 1376 out rows per core
KP = IN // 2                 # 2048 packed bytes per out row
NCH = KP // 128              # 16 weight chunks of 128 partitions
NGRP = NCH // 4              # 4 chunk quads (one DMA + one DVE pass each)
OBLOCKS = [(0, 512), (512, 512), (1024, OSH - 1024)]

# Engine split knobs.
HS_GP_PAIRS = 0      # of 8 hs-mult pairs, how many run on GPSIMD
TRANSPORT_BF16 = False  # bf16 transport needs float mod (ISA-illegal on TS path)
H_ON_ACT = True      # h-extract via ACT Copy(scale=1/16, bias=ACT_BIAS) -> int16
ACT_BIAS = -0.46875  # -7.5/16 for round-to-nearest

_prog_cache = {}


def build_program(n_loop=None):
    """Build the bass program. n_loop=None -> single shot (graded path);
    n_loop=N wraps the whole body in a hardware For_i for slope timing."""
    nc = bacc.Bacc("TRN2", target_bir_lowering=False)

    wdt = dt.bfloat16 if TRANSPORT_BF16 else dt.int16
    wP = nc.declare_dram_parameter("wP", [128, NGRP * 4 * OSH], wdt, isOutput=False)
    xte = nc.declare_dram_parameter("xte", [128, NCH * B], dt.bfloat16, isOutput=False)
    xto = nc.declare_dram_parameter("xto", [128, NCH * B], dt.bfloat16, isOutput=False)
    sce = nc.declare_dram_parameter("sce", [128, 4 * OSH], dt.bfloat16, isOutput=False)
    sco = nc.declare_dram_parameter("sco", [128, 4 * OSH], dt.bfloat16, isOutput=False)
    tT = nc.declare_dram_parameter("tT", [128, OSH], dt.float32, isOutput=False)
    xs = nc.declare_dram_parameter("xs", [128, B], dt.float32, isOutput=False)
    bias = nc.declare_dram_parameter("bias", [1, OSH], dt.float32, isOutput=False)
    ones = nc.declare_dram_parameter("ones", [1, B], dt.float32, isOutput=False)
    y = nc.declare_dram_parameter("y", [B, OSH], dt.float32, isOutput=True)

    import contextlib

    with tile.TileContext(nc) as tc, contextlib.ExitStack() as _loop:
        with (
            tc.tile_pool(name="const", bufs=1) as cpool,
            tc.tile_pool(name="w", bufs=2) as wpool,
            tc.tile_pool(name="dq", bufs=2) as dqpool,
            tc.tile_pool(name="ps", bufs=1, space="PSUM") as pspool,
            tc.tile_pool(name="out", bufs=2) as opool,
        ):
            xte_sb = cpool.tile([128, NCH * B], dt.bfloat16, tag="xte")
            nc.sync.dma_start(out=xte_sb[:], in_=xte[:])
            xto_sb = cpool.tile([128, NCH * B], dt.bfloat16, tag="xto")
            nc.sync.dma_start(out=xto_sb[:], in_=xto[:])
            sce_sb = cpool.tile([128, 4 * OSH], dt.bfloat16, tag="sce")
            nc.sync.dma_start(out=sce_sb[:], in_=sce[:])
            sco_sb = cpool.tile([128, 4 * OSH], dt.bfloat16, tag="sco")
            nc.sync.dma_start(out=sco_sb[:], in_=sco[:])
            tT_sb = cpool.tile([128, OSH], dt.float32, tag="tT")
            nc.sync.dma_start(out=tT_sb[:], in_=tT[:])
            xs_sb = cpool.tile([128, B], dt.float32, tag="xs")
            nc.sync.dma_start(out=xs_sb[:], in_=xs[:])
            bias_sb = cpool.tile([1, OSH], dt.float32, tag="bias")
            nc.sync.dma_start(out=bias_sb[:], in_=bias[:])
            ones_sb = cpool.tile([1, B], dt.float32, tag="ones")
            nc.sync.dma_start(out=ones_sb[:], in_=ones[:])

            if n_loop:
                _loop.enter_context(tc.For_i(0, n_loop, 1))

            psums = []
            for o0, ow in OBLOCKS:
                ps = pspool.tile([B, ow], dt.float32, tag=f"ps{o0}")
                nc.tensor.matmul(
                    ps[:], ones_sb[:], bias_sb[:, o0 : o0 + ow],
                    start=True, stop=False,
                )
                nc.tensor.matmul(
                    ps[:], xs_sb[:], tT_sb[:, o0 : o0 + ow],
                    start=False, stop=False,
                )
                psums.append(ps)

            gp_pairs = (
                set(round(g * NGRP / HS_GP_PAIRS) for g in range(HS_GP_PAIRS))
                if HS_GP_PAIRS
                else set()
            )
            W2 = 4 * OSH
            for g in range(NGRP):
                tb = wpool.tile([128, W2], wdt, tag="tb")
                nc.sync.dma_start(out=tb[:], in_=wP[:, g * W2 : (g + 1) * W2])
                h16 = dqpool.tile([128, W2], dt.int16, tag="h16")
                if H_ON_ACT:
                    nc.scalar.activation(
                        h16[:], tb[:], mybir.ActivationFunctionType.Copy,
                        bias=ACT_BIAS, scale=0.0625,
                    )
                else:
                    nc.vector.tensor_scalar(
                        h16[:], tb[:], 4, None, Alu.logical_shift_right
                    )
                if TRANSPORT_BF16:
                    l16 = dqpool.tile([128, W2], dt.bfloat16, tag="l16")
                    nc.vector.tensor_scalar(l16[:], tb[:], 16.0, None, Alu.mod)
                else:
                    l16 = dqpool.tile([128, W2], dt.int16, tag="l16")
                    nc.vector.tensor_scalar(l16[:], tb[:], 15, None, Alu.bitwise_and)
                hs = dqpool.tile([128, W2], dt.bfloat16, tag="hs")
                ls = dqpool.tile([128, W2], dt.bfloat16, tag="ls")
                mh_eng = nc.gpsimd if g in gp_pairs else nc.vector
                mh_eng.tensor_tensor(hs[:], h16[:], sce_sb[:], Alu.mult)
                nc.vector.tensor_tensor(ls[:], l16[:], sco_sb[:], Alu.mult)
                for q in range(4):
                    c = 4 * g + q
                    last = c == NCH - 1
                    for i, (o0, ow) in enumerate(OBLOCKS):
                        nc.tensor.matmul(
                            psums[i][:],
                            xte_sb[:, c * B : (c + 1) * B],
                            hs[:, q * OSH + o0 : q * OSH + o0 + ow],
                            start=False, stop=False,
                        )
                        nc.tensor.matmul(
                            psums[i][:],
                            xto_sb[:, c * B : (c + 1) * B],
                            ls[:, q * OSH + o0 : q * OSH + o0 + ow],
                            start=False, stop=last,
                        )

            for i, (o0, ow) in enumerate(OBLOCKS):
                ot = opool.tile([B, ow], dt.float32, tag=f"ot{i}")
                nc.scalar.copy(out=ot[:], in_=psums[i][:])
                nc.sync.dma_start(out=y[:, o0 : o0 + ow], in_=ot[:])

    nc.compile()
    return nc


def prep_core_inputs(x, weight, scale, zp, bias):
    """Build the per-core input maps (numpy layout shuffles only)."""
    bf16 = ml_dtypes.bfloat16
    x = np.asarray(x, dtype=np.float32)
    weight = np.asarray(weight, dtype=np.int32)
    scale = np.asarray(scale, dtype=np.float32)
    zp = np.asarray(zp, dtype=np.float32)
    bias = np.asarray(bias, dtype=np.float32)

    # packed byte per int32, pre-transposed per core
    w8 = weight.astype(np.uint8)  # [OUT, KP], values 0..255

    # x columns arranged to the chunk layout: chunk c, partition p
    # <-> (i_hi, i_lo) = (2*(128c+p), 2*(128c+p)+1)
    xT = x.T  # [IN, B]
    xe = xT[0::2].reshape(NCH, 128, B).transpose(1, 0, 2).reshape(128, NCH * B)
    xo = xT[1::2].reshape(NCH, 128, B).transpose(1, 0, 2).reshape(128, NCH * B)
    xte_h = np.ascontiguousarray(xe).astype(bf16)
    xto_h = np.ascontiguousarray(xo).astype(bf16)

    xs_h = np.ascontiguousarray(x.reshape(B, IN // BLK, BLK).sum(axis=1).T)  # [128,B]
    ones_h = np.ones((1, B), dtype=np.float32)

    in_maps = []
    for c in range(NCORES):
        rows = slice(c * OSH, (c + 1) * OSH)
        wdt_np = ml_dtypes.bfloat16 if TRANSPORT_BF16 else np.int16
        wT_h = w8[rows].T.astype(wdt_np)  # [KP, OSH]
        # quad-group layout: wP[p, 5504 g + 1376 q + j] = wT[512g + 128q + p, j]
        wP_h = np.ascontiguousarray(
            wT_h.reshape(NGRP, 4, 128, OSH).transpose(2, 0, 1, 3).reshape(
                128, NGRP * 4 * OSH
            )
        )
        s_c = scale[rows, 0, :]  # [OSH, 128]
        z_c = zp[rows, 0, :]
        sce1 = np.tile(s_c[:, 0::2].T, (2, 1))  # [128, OSH]
        sco1 = np.tile(s_c[:, 1::2].T, (2, 1))
        sce_h = np.ascontiguousarray(np.tile(sce1, (1, 4))).astype(bf16)
        sco_h = np.ascontiguousarray(np.tile(sco1, (1, 4))).astype(bf16)
        tT_h = np.ascontiguousarray(-(s_c * z_c).T)  # [128, OSH] f32
        bias_h = np.ascontiguousarray(bias[rows]).reshape(1, OSH)
        in_maps.append(
            {
                "wP": wP_h,
                "xte": xte_h,
                "xto": xto_h,
                "sce": sce_h,
                "sco": sco_h,
                "tT": tT_h,
                "xs": xs_h,
                "bias": bias_h,
                "ones": ones_h,
            }
        )
    return in_maps


def kernel(x, weight, scale, zp, bias):
    if "nc" not in _prog_cache:
        _prog_cache["nc"] = build_program()
    nc = _prog_cache["nc"]
    in_maps = prep_core_inputs(x, weight, scale, zp, bias)
    res = run_bass_kernel_spmd(nc, in_maps, core_ids=list(range(NCORES)))
    shards = [res.results[c]["y"] for c in range(NCORES)]
    return np.concatenate(shards, axis=1).astype(np.float32)


# revision 13
# speedup vs baseline: 1.0911x; 1.0911x over previous
"""Trainium2 Bass kernel: 4-bit block-dequant linear  y = x @ dequant(W).T + bias.

Shapes (hardcoded): x[64,4096] f32, weight[11008,2048] int32 (two uint4 nibbles
in the low byte of each int32), scale/zp[11008,1,128] f32, bias[11008] f32.
Output y[64,11008] f32.

Strategy (8-way tensor-parallel over out_features, 1376 rows per core):

  y[b,o] = sum_c x[b,c] * w[o,c] * s[o, c%128]
         - sum_j (zp[o,j]*s[o,j]) * xs[b,j]          (zero-point correction)
         + bias[o]
  where xs[b,j] = sum_i x[b, 128i+j].

Host-side prep: the packed weight is cast to its information content (one
byte per int32, value 0..255, as int16) and laid out per core as 8 pair-groups
of 2x128 partition-rows: wP[p, 2752*g + 1376*q + j] = byte k=256g+128q+p of
out-row j.  On device, per core, 8 pair-groups:
  * one contiguous-per-partition DMA of tb[128, 2752] int16
  * h16 = tb >> 4 (ACT Copy(scale=1/16, bias=-7.5/16) -> int16 rounds = floor)
  * l   = tb & 15 (DVE tensor_scalar 4x-ish, bf16 out)
  * hs = h16 * sce2 (bf16), ls = l * sco2  (DVE 2x / GPSIMD split)
  * PE accumulates into 3 PSUM tiles [64, o-block<=512]:
      bias (K=1) + zp-correction (K=128, rhs = -(zp*s).T)
      + 2x2x3 bf16 matmuls (lhsT = matching x columns, rhs = hs/ls slices)
  * ACT evicts PSUM -> SBUF, DMA to DRAM.

Partition p of pair g / sub q holds byte k = 256g+128q+p = column pair
(i_hi, i_lo) = (2k, 2k+1); i_hi % 128 = 2*(p%64), so the scale tiles
sce[p,o] = s[o, 2(p%64)], sco[p,o] = s[o, 2(p%64)+1] are chunk-independent.
"""

import sys

import numpy as np

for _p in ("/opt/trn_rl_repo", "/root/.axon_site/_ro/trn_rl_repo"):
    if _p not in sys.path:
        sys.path.insert(0, _p)

import ml_dtypes  # noqa: E402
import concourse.bass as bass  # noqa: E402
import concourse.bacc as bacc  # noqa: E402
import concourse.mybir as mybir  # noqa: E402
from concourse import tile  # noqa: E402
from concourse.bass_utils import run_bass_kernel_spmd  # noqa: E402

dt = mybir.dt
Alu = mybir.AluOpType

B = 64
IN = 4096
OUT = 11008
BLK = 128
NCORES = 8
OSH = OUT // NCORES          # 1376 out rows per core
KP = IN // 2                 # 2048 packed bytes per out row
NCH = KP // 128              # 16 weight chunks of 128 partitions
NPAIR = NCH // 2             # 8 chunk pairs (one DMA + one DVE pass each)
OBLOCKS = [(0, 512), (512, 512), (1024, OSH - 1024)]

# Engine split knobs.
HS_GP_PAIRS = 0      # of 8 hs-mult pairs, how many run on GPSIMD
TRANSPORT_BF16 = False  # bf16 transport needs float mod (ISA-illegal on TS path)
H_ON_ACT = True      # h-extract via ACT Copy(scale=1/16, bias=ACT_BIAS) -> int16
ACT_BIAS = -0.46875  # -7.5/16 for round-to-nearest

_prog_cache = {}


def build_program(n_loop=None):
    """Build the bass program. n_loop=None -> single shot (graded path);
    n_loop=N wraps the whole body in a hardware For_i for slope timing."""
    nc = bacc.Bacc("TRN2", target_bir_lowering=False)

    wdt = dt.bfloat16 if TRANSPORT_BF16 else dt.int16
    wP = nc.declare_dram_parameter("wP", [128, NPAIR * 2 * OSH], wdt, isOutput=False)
    xte = nc.declare_dram_parameter("xte", [128, NCH * B], dt.bfloat16, isOutput=False)
    xto = nc.declare_dram_parameter("xto", [128, NCH * B], dt.bfloat16, isOutput=False)
    sce = nc.declare_dram_parameter("sce", [128, 2 * OSH], dt.bfloat16, isOutput=False)
    sco = nc.declare_dram_parameter("sco", [128, 2 * OSH], dt.bfloat16, isOutput=False)
    tT = nc.declare_dram_parameter("tT", [128, OSH], dt.float32, isOutput=False)
    xs = nc.declare_dram_parameter("xs", [128, B], dt.float32, isOutput=False)
    bias = nc.declare_dram_parameter("bias", [1, OSH], dt.float32, isOutput=False)
    ones = nc.declare_dram_parameter("ones", [1, B], dt.float32, isOutput=False)
    y = nc.declare_dram_parameter("y", [B, OSH], dt.float32, isOutput=True)

    import contextlib

    with tile.TileContext(nc) as tc, contextlib.ExitStack() as _loop:
        with (
            tc.tile_pool(name="const", bufs=1) as cpool,
            tc.tile_pool(name="w", bufs=4) as wpool,
            tc.tile_pool(name="dq", bufs=4) as dqpool,
            tc.tile_pool(name="ps", bufs=1, space="PSUM") as pspool,
            tc.tile_pool(name="out", bufs=2) as opool,
        ):
            xte_sb = cpool.tile([128, NCH * B], dt.bfloat16, tag="xte")
            nc.sync.dma_start(out=xte_sb[:], in_=xte[:])
            xto_sb = cpool.tile([128, NCH * B], dt.bfloat16, tag="xto")
            nc.sync.dma_start(out=xto_sb[:], in_=xto[:])
            sce_sb = cpool.tile([128, 2 * OSH], dt.bfloat16, tag="sce")
            nc.sync.dma_start(out=sce_sb[:], in_=sce[:])
            sco_sb = cpool.tile([128, 2 * OSH], dt.bfloat16, tag="sco")
            nc.sync.dma_start(out=sco_sb[:], in_=sco[:])
            tT_sb = cpool.tile([128, OSH], dt.float32, tag="tT")
            nc.sync.dma_start(out=tT_sb[:], in_=tT[:])
            xs_sb = cpool.tile([128, B], dt.float32, tag="xs")
            nc.sync.dma_start(out=xs_sb[:], in_=xs[:])
            bias_sb = cpool.tile([1, OSH], dt.float32, tag="bias")
            nc.sync.dma_start(out=bias_sb[:], in_=bias[:])
            ones_sb = cpool.tile([1, B], dt.float32, tag="ones")
            nc.sync.dma_start(out=ones_sb[:], in_=ones[:])

            if n_loop:
                _loop.enter_context(tc.For_i(0, n_loop, 1))

            psums = []
            for o0, ow in OBLOCKS:
                ps = pspool.tile([B, ow], dt.float32, tag=f"ps{o0}")
                nc.tensor.matmul(
                    ps[:], ones_sb[:], bias_sb[:, o0 : o0 + ow],
                    start=True, stop=False,
                )
                nc.tensor.matmul(
                    ps[:], xs_sb[:], tT_sb[:, o0 : o0 + ow],
                    start=False, stop=False,
                )
                psums.append(ps)

            gp_pairs = (
                set(round(g * NPAIR / HS_GP_PAIRS) for g in range(HS_GP_PAIRS))
                if HS_GP_PAIRS
                else set()
            )
            W2 = 2 * OSH
            for g in range(NPAIR):
                tb = wpool.tile([128, W2], wdt, tag="tb")
                nc.sync.dma_start(out=tb[:], in_=wP[:, g * W2 : (g + 1) * W2])
                h16 = dqpool.tile([128, W2], dt.int16, tag="h16")
                if H_ON_ACT:
                    nc.scalar.activation(
                        h16[:], tb[:], mybir.ActivationFunctionType.Copy,
                        bias=ACT_BIAS, scale=0.0625,
                    )
                else:
                    nc.vector.tensor_scalar(
                        h16[:], tb[:], 4, None, Alu.logical_shift_right
                    )
                if TRANSPORT_BF16:
                    l16 = dqpool.tile([128, W2], dt.bfloat16, tag="l16")
                    nc.vector.tensor_scalar(l16[:], tb[:], 16.0, None, Alu.mod)
                else:
                    l16 = dqpool.tile([128, W2], dt.int16, tag="l16")
                    nc.vector.tensor_scalar(l16[:], tb[:], 15, None, Alu.bitwise_and)
                hs = dqpool.tile([128, W2], dt.bfloat16, tag="hs")
                ls = dqpool.tile([128, W2], dt.bfloat16, tag="ls")
                mh_eng = nc.gpsimd if g in gp_pairs else nc.vector
                mh_eng.tensor_tensor(hs[:], h16[:], sce_sb[:], Alu.mult)
                nc.vector.tensor_tensor(ls[:], l16[:], sco_sb[:], Alu.mult)
                for q in range(2):
                    c = 2 * g + q
                    last = c == NCH - 1
                    for i, (o0, ow) in enumerate(OBLOCKS):
                        nc.tensor.matmul(
                            psums[i][:],
                            xte_sb[:, c * B : (c + 1) * B],
                            hs[:, q * OSH + o0 : q * OSH + o0 + ow],
                            start=False, stop=False,
                        )
                        nc.tensor.matmul(
                            psums[i][:],
                            xto_sb[:, c * B : (c + 1) * B],
                            ls[:, q * OSH + o0 : q * OSH + o0 + ow],
                            start=False, stop=last,
                        )

            for i, (o0, ow) in enumerate(OBLOCKS):
                ot = opool.tile([B, ow], dt.float32, tag=f"ot{i}")
                nc.scalar.copy(out=ot[:], in_=psums[i][:])
                nc.sync.dma_start(out=y[:, o0 : o0 + ow], in_=ot[:])

    nc.compile()
    return nc


def prep_core_inputs(x, weight, scale, zp, bias):
    """Build the per-core input maps (numpy layout shuffles only)."""
    bf16 = ml_dtypes.bfloat16
    x = np.asarray(x, dtype=np.float32)
    weight = np.asarray(weight, dtype=np.int32)
    scale = np.asarray(scale, dtype=np.float32)
    zp = np.asarray(zp, dtype=np.float32)
    bias = np.asarray(bias, dtype=np.float32)

    # packed byte per int32, pre-transposed per core
    w8 = weight.astype(np.uint8)  # [OUT, KP], values 0..255

    # x columns arranged to the chunk layout: chunk c, partition p
    # <-> (i_hi, i_lo) = (2*(128c+p), 2*(128c+p)+1)
    xT = x.T  # [IN, B]
    xe = xT[0::2].reshape(NCH, 128, B).transpose(1, 0, 2).reshape(128, NCH * B)
    xo = xT[1::2].reshape(NCH, 128, B).transpose(1, 0, 2).reshape(128, NCH * B)
    xte_h = np.ascontiguousarray(xe).astype(bf16)
    xto_h = np.ascontiguousarray(xo).astype(bf16)

    xs_h = np.ascontiguousarray(x.reshape(B, IN // BLK, BLK).sum(axis=1).T)  # [128,B]
    ones_h = np.ones((1, B), dtype=np.float32)

    in_maps = []
    for c in range(NCORES):
        rows = slice(c * OSH, (c + 1) * OSH)
        wdt_np = ml_dtypes.bfloat16 if TRANSPORT_BF16 else np.int16
        wT_h = w8[rows].T.astype(wdt_np)  # [KP, OSH]
        # pair-group layout: wP[p, 2752 g + 1376 q + j] = wT[256g + 128q + p, j]
        wP_h = np.ascontiguousarray(
            wT_h.reshape(NPAIR, 2, 128, OSH).transpose(2, 0, 1, 3).reshape(
                128, NPAIR * 2 * OSH
            )
        )
        s_c = scale[rows, 0, :]  # [OSH, 128]
        z_c = zp[rows, 0, :]
        sce1 = np.tile(s_c[:, 0::2].T, (2, 1))  # [128, OSH]
        sco1 = np.tile(s_c[:, 1::2].T, (2, 1))
        sce_h = np.ascontiguousarray(np.tile(sce1, (1, 2))).astype(bf16)
        sco_h = np.ascontiguousarray(np.tile(sco1, (1, 2))).astype(bf16)
        tT_h = np.ascontiguousarray(-(s_c * z_c).T)  # [128, OSH] f32
        bias_h = np.ascontiguousarray(bias[rows]).reshape(1, OSH)
        in_maps.append(
            {
                "wP": wP_h,
                "xte": xte_h,
                "xto": xto_h,
                "sce": sce_h,
                "sco": sco_h,
                "tT": tT_h,
                "xs": xs_h,
                "bias": bias_h,
                "ones": ones_h,
            }
        )
    return in_maps


def kernel(x, weight, scale, zp, bias):
    if "nc" not in _prog_cache:
        _prog_cache["nc"] = build_program()
    nc = _prog_cache["nc"]
    in_maps = prep_core_inputs(x, weight, scale, zp, bias)
    res = run_bass_kernel_spmd(nc, in_maps, core_ids=list(range(NCORES)))
    shards = [res.results[c]["y"] for c in range(NCORES)]
    return np.concatenate(shards, axis=1).astype(np.float32)
